# revision 5
# baseline (speedup 1.0000x reference)
"""GATConv-style message passing kernel for Trainium2 (8 NeuronCores).

Math (reference):
    h   = feat @ W + b                      # [N, F]
    m   = h[src] * att                      # [E, F]
    out = segment_sum(m, dst, N)            # [N, F]

Device strategy (dst-sharded, no collectives needed):
  - Each of the 8 cores owns an output-node range of NPC = N/8 nodes and
    receives ALL edges whose dst falls in its range (host buckets + sorts).
  - Aggregation is done on raw features (linearity): agg = segsum(att*feat[src]),
    then out = agg @ W on-device; the bias term (segsum(att) outer b) is added
    on the host (b is zeros for this problem, handled generally anyway).
  - Per core the edges are packed into windows of S=128 consecutive output
    nodes, C chunks of 128 edges per window (padded with att=0 edges).
    Per chunk: indirect-DMA gather of feat[src] -> [128,64], one fused DVE
    tensor_scalar builds the att-scaled one-hot [128 edges, 128 slots]
    (is_equal vs iota, then * att), and a matmul msg^T @ onehot accumulates
    [64, 128] into a PSUM window. Windows flush to an SBUF staging tensor
    [64, 12544] (feature-major), which feeds the final W matmuls directly
    as pre-transposed lhsT.
"""
import sys
sys.path.insert(0, "/opt/trn_rl_repo")

from contextlib import ExitStack

import numpy as np

import concourse.bass as bass
import concourse.mybir as mybir
import concourse.tile as tile
from concourse import bacc
from concourse.bass_utils import run_bass_kernel_spmd

P = 128
F = 64
N_NODES = 100000
N_EDGES = 1600000
N_CORES = 8
NPC = N_NODES // N_CORES          # 12500 output nodes per core
S = 128                           # window width (output-node slots)
NW = (NPC + S - 1) // S           # 98 windows per core
NSLOT = NW * S                    # 12544 staged node slots per core
NT = NSLOT // P                   # 98 final matmul tiles

_program_cache = {}


def _build_program(C: int):
    """Build the SPMD Bass program for C chunks per window."""
    CH = NW * C
    nc = bacc.Bacc(trn_type="TRN2", target_bir_lowering=False, debug=False,
                   num_devices=N_CORES)

    featD = nc.dram_tensor("feat", [N_NODES, F], mybir.dt.float32,
                           kind="ExternalInput")
    gidxD = nc.dram_tensor("gidx", [P, CH], mybir.dt.int32,
                           kind="ExternalInput")
    sclD = nc.dram_tensor("scl", [P, 2 * CH], mybir.dt.float32,
                          kind="ExternalInput")
    wD = nc.dram_tensor("w", [F, F], mybir.dt.float32, kind="ExternalInput")
    outD = nc.dram_tensor("out", [P, NT * F], mybir.dt.float32,
                          kind="ExternalOutput")

    with tile.TileContext(nc) as tc, ExitStack() as ctx:
        const = ctx.enter_context(tc.tile_pool(name="const", bufs=1))
        meta = ctx.enter_context(tc.tile_pool(name="meta", bufs=1))
        msgs = ctx.enter_context(tc.tile_pool(name="msgs", bufs=12))
        ohp = ctx.enter_context(tc.tile_pool(name="ohp", bufs=8))
        stag = ctx.enter_context(tc.tile_pool(name="stag", bufs=1))
        psum = ctx.enter_context(tc.tile_pool(name="psum", bufs=4,
                                              space="PSUM"))
        psum2 = ctx.enter_context(tc.tile_pool(name="psum2", bufs=2,
                                               space="PSUM"))

        iota_i = const.tile([P, S], mybir.dt.int32)
        nc.gpsimd.iota(iota_i[:], pattern=[[1, S]], base=0,
                       channel_multiplier=0)
        iota_f = const.tile([P, S], mybir.dt.float32)
        nc.vector.tensor_copy(iota_f[:], iota_i[:])

        wsb = const.tile([F, F], mybir.dt.float32)
        nc.sync.dma_start(wsb[:], wD[:])

        gidx = meta.tile([P, CH], mybir.dt.int32)
        nc.sync.dma_start(gidx[:], gidxD[:])
        scl = meta.tile([P, 2 * CH], mybir.dt.float32)
        nc.sync.dma_start(scl[:], sclD[:])
        dstl = scl[:, 0:CH]
        attv = scl[:, CH:2 * CH]

        staging = stag.tile([F, NSLOT], mybir.dt.float32)

        for w in range(NW):
            ps = psum.tile([F, S], mybir.dt.float32, name="ps", tag="ps")
            for c in range(C):
                k = w * C + c
                msg = msgs.tile([P, F], mybir.dt.float32, name="msg",
                                tag="msg")
                nc.gpsimd.indirect_dma_start(
                    out=msg[:],
                    out_offset=None,
                    in_=featD[:],
                    in_offset=bass.IndirectOffsetOnAxis(
                        ap=gidx[:, k:k + 1], axis=0),
                )
                oh = ohp.tile([P, S], mybir.dt.float32, name="oh", tag="oh")
                nc.vector.tensor_scalar(
                    out=oh[:],
                    in0=iota_f[:],
                    scalar1=dstl[:, k:k + 1],
                    scalar2=attv[:, k:k + 1],
                    op0=mybir.AluOpType.is_equal,
                    op1=mybir.AluOpType.mult,
                )
                nc.tensor.matmul(
                    out=ps[:], lhsT=msg[:], rhs=oh[:],
                    start=(c == 0), stop=(c == C - 1),
                )
            nc.vector.tensor_copy(staging[:, w * S:(w + 1) * S], ps[:])

        out_sb = stag.tile([P, NT * F], mybir.dt.float32)
        for t in range(NT):
            ps2 = psum2.tile([P, F], mybir.dt.float32, name="fin", tag="fin")
            nc.tensor.matmul(
                out=ps2[:],
                lhsT=staging[:, t * P:(t + 1) * P],
                rhs=wsb[:],
                start=True, stop=True,
            )
            nc.vector.tensor_copy(out_sb[:, t * F:(t + 1) * F], ps2[:])
        nc.sync.dma_start(outD[:], out_sb[:])

    nc.finalize()
    return nc


def _host_pack(src, dst, attention):
    """Bucket edges by owning core, sort by dst, pack into window/chunk/lane
    layout. Returns per-core (gidx[P,CH], dstl[P,CH], attv[P,CH]) and C."""
    att = attention.reshape(-1)
    order = np.argsort(dst, kind="stable")
    src_s = src[order]
    dst_s = dst[order]
    att_s = att[order]

    # window boundaries in sorted-dst space for every (core, window)
    bounds = []
    for q in range(N_CORES):
        base = q * NPC
        ws = np.minimum(base + np.arange(NW + 1) * S, base + NPC)
        bounds.append(ws)
    bounds = np.concatenate(bounds)                      # [N_CORES*(NW+1)]
    idx = np.searchsorted(dst_s, bounds)
    idx = idx.reshape(N_CORES, NW + 1)

    counts = idx[:, 1:] - idx[:, :-1]                    # [N_CORES, NW]
    max_edges = counts.max() if counts.size else 0
    C = max(int(-(-max_edges // P)), 1)
    CH = NW * C

    # position of each edge within its window
    win_of_edge_start = idx[:, :-1]                      # start index per (q,w)
    # build per-edge window start via repeat
    starts_flat = np.repeat(win_of_edge_start.reshape(-1),
                            counts.reshape(-1))
    pos = np.arange(len(dst_s)) - starts_flat            # position in window
    chunk = pos // P
    lane = pos % P
    # global (q, w) per edge
    qw = np.repeat(np.arange(N_CORES * NW), counts.reshape(-1))
    q_of_edge = qw // NW
    w_of_edge = qw % NW
    slot = (w_of_edge * C + chunk).astype(np.int64)      # chunk id within core

    gidx = np.zeros((N_CORES, CH, P), dtype=np.int32)
    dstl = np.zeros((N_CORES, CH, P), dtype=np.float32)
    attv = np.zeros((N_CORES, CH, P), dtype=np.float32)

    gidx[q_of_edge, slot, lane] = src_s
    dstl[q_of_edge, slot, lane] = (
        dst_s - q_of_edge * NPC - w_of_edge * S).astype(np.float32)
    attv[q_of_edge, slot, lane] = att_s

    # partition-major for direct DMA: [P, CH]; scl = [dstl | attv] on free dim
    gidx = np.ascontiguousarray(gidx.transpose(0, 2, 1))
    scl = np.concatenate(
        [dstl.transpose(0, 2, 1), attv.transpose(0, 2, 1)], axis=2)
    scl = np.ascontiguousarray(scl)
    return gidx, scl, C


def _run(feat, attention, src, dst, weight, h_bias, trace=False):
    feat = np.ascontiguousarray(feat, dtype=np.float32)
    weight = np.ascontiguousarray(weight, dtype=np.float32)
    gidx, scl, C = _host_pack(src, dst, attention)

    if C not in _program_cache:
        _program_cache[C] = _build_program(C)
    nc = _program_cache[C]

    in_maps = []
    for q in range(N_CORES):
        in_maps.append({
            "feat": feat,
            "gidx": gidx[q],
            "scl": scl[q],
            "w": weight,
        })
    res = run_bass_kernel_spmd(nc, in_maps, core_ids=list(range(N_CORES)),
                               trace=trace)

    out = np.empty((N_NODES, F), dtype=np.float32)
    for q in range(N_CORES):
        o = res.results[q]["out"]                        # [P, NT*F]
        o = o.reshape(P, NT, F).transpose(1, 0, 2).reshape(NSLOT, F)
        out[q * NPC:(q + 1) * NPC] = o[:NPC]

    # bias term: out += segsum(att, dst) * b  (b is zeros for this problem)
    if np.any(h_bias):
        deg_att = np.bincount(
            dst, weights=attention.reshape(-1),
            minlength=N_NODES).astype(np.float32)
        out += deg_att[:, None] * h_bias[None, :].astype(np.float32)

    return out, res


def kernel(feat, attention, src, dst, weight, h_bias):
    out, _ = _run(feat, attention, src, dst, weight, h_bias)
    return out
